# revision 1
# baseline (speedup 1.0000x reference)
"""Dilated-attention (segmented FlashMHA) for Trainium2, 8-core data parallel.

Problem (hardcoded): x [2, 8192, 1024], SEGMENT=2048, DILATION=2, 16 heads.
Each (batch, segment) pair is an independent attention problem over the
L = 1024 dilated tokens; there are exactly B * n_seg = 2 * 4 = 8 of them,
one per NeuronCore.  Weights are replicated.

Per-core kernel (all matmuls float32r = full PE rate at N>=256):
  xsT  = transpose(xs)                     PE transpose, 64 128x128 blocks
  qkT  = Wqkv[:, :2048].T @ xsT  (+bias)   q/k kept transposed [dim, token]
  v    = xs @ Wqkv[:, 2048:]    (+bias)    natural [token, dim], stored
                                           head-blocked with a ones column
                                           appended per head (v_aug)
  per head:  sT = k.q (transposed scores), eT = exp(sT/8) via ACT,
             ctxT_aug = sum_ck v_aug.T-contract @ eT  ([65, lq]; row 64 is
             the softmax denominator via the ones column),
             ctxT = ctxT_aug[0:64] * recip(denom) (DVE + gpsimd bcast)
  out  = ctxT.T-contract @ Wout + bout     natural layout

HW-tuning notes (all A/B measured on device):
- kT is stored one head per zero-padded 128-row tile (dead rows zeroed
  by a masked tensor_scalar eviction) so score matmuls contract K=128
  from base partition 0: partial-K (64) matmuls measured ~2.2x slower,
  and alternating operand base partitions between adjacent matmuls is
  also ~2x slower.
- The per-head attention loop is emitted interleaved with v halves and
  later heads' q/k projection tiles (fillers) so the PE stays busy while
  ACT runs the exps; ctx PSUM is single-buffered with an immediate DVE
  eviction (craw) and the softmax normalization runs asynchronously.
"""

from contextlib import ExitStack

import numpy as np

from concourse import bacc, bass_utils, mybir, tile
from concourse._compat import with_exitstack
from concourse.masks import make_identity

F32 = mybir.dt.float32
F32R = mybir.dt.float32r
BF16 = mybir.dt.bfloat16
AF = mybir.ActivationFunctionType

B = 2
S = 8192
D = 1024
SEGMENT = 2048
DILATION = 2
N_SEG = S // SEGMENT          # 4
L = SEGMENT // DILATION       # 1024 tokens per (b, seg)
H = 16
HD = 64
NQK = 2048
SCALE = 0.125                 # 1 / sqrt(HD)
N_CORES = 8

_CACHE = {}


def _build(n_cores=N_CORES, loop_n=1):
    nc = bacc.Bacc("TRN2", debug=False, num_devices=n_cores)

    xs_d = nc.dram_tensor("xs", (L, D), F32, kind="ExternalInput")
    wqkv_d = nc.dram_tensor("Wqkv", (D, 3 * D), F32, kind="ExternalInput")
    bqkv_d = nc.dram_tensor("bqkv", (3 * D,), F32, kind="ExternalInput")
    wout_d = nc.dram_tensor("Wout", (D, D), F32, kind="ExternalInput")
    bout_d = nc.dram_tensor("bout", (D,), F32, kind="ExternalInput")
    out_d = nc.dram_tensor("out", (L, D), F32, kind="ExternalOutput")

    with tile.TileContext(nc) as tc:
        if loop_n > 1:
            with tc.For_i(0, loop_n, 1):
                _emit(tc, out_d.ap(), xs_d.ap(), wqkv_d.ap(), bqkv_d.ap(),
                      wout_d.ap(), bout_d.ap())
        else:
            _emit(tc, out_d.ap(), xs_d.ap(), wqkv_d.ap(), bqkv_d.ap(),
                  wout_d.ap(), bout_d.ap())
    nc.compile()
    return nc


@with_exitstack
def _emit(ctx: ExitStack, tc, out, xs, wqkv, bqkv, wout, bout):
    nc = tc.nc

    const_p = ctx.enter_context(tc.tile_pool(name="const", bufs=1))
    ctxT_p = ctx.enter_context(tc.tile_pool(name="ctxT", bufs=8))
    # PSUM pool shared by all projection matmuls (qk / v / out)
    proj_ps = ctx.enter_context(
        tc.tile_pool(name="proj_ps", bufs=2, space="PSUM"))

    # f32 constants: identity (transpose) at cols 0:128, all-ones 128:256
    cst = const_p.tile([128, 256], F32)
    make_identity(nc, cst[:, 0:128])
    nc.vector.memset(cst[:, 128:256], 1.0)
    identity = cst[:, 0:128]
    ones16 = cst[:, 128:144]
    bqk = const_p.tile([128, 16], F32)
    nc.sync.dma_start(out=bqk[:], in_=bqkv[0:NQK].rearrange("(c p) -> p c", p=128))
    # bias tiles broadcast across partitions (0-stride DRAM read)
    bv_bc = const_p.tile([128, D], F32)
    nc.gpsimd.dma_start(out=bv_bc[:],
                        in_=bqkv[NQK:3 * D].partition_broadcast(128))
    bout_bc = const_p.tile([128, D], F32)
    nc.gpsimd.dma_start(out=bout_bc[:], in_=bout.partition_broadcast(128))
    idr = const_p.tile([128, 128], F32R)
    nc.vector.tensor_copy(out=idr[:], in_=cst[:, 0:128])
    # partition masks: col 0 selects rows 0:64, col 1 selects rows 64:128
    pmask = const_p.tile([128, 2], F32)
    nc.vector.memset(pmask[0:HD, 0:1], 1.0)
    nc.vector.memset(pmask[HD:128, 0:1], 0.0)
    nc.vector.memset(pmask[0:HD, 1:2], 0.0)
    nc.vector.memset(pmask[HD:128, 1:2], 1.0)

    ctxT = [ctxT_p.tile([128, L], F32R, tag="ctxT", name=f"ctxT{i}")
            for i in range(8)]

    with tc.tile_pool(name="xsT", bufs=8) as xsT_p, \
         tc.tile_pool(name="vaug", bufs=8) as vaug_p, \
         tc.tile_pool(name="qkT", bufs=6) as qkT_p, \
         tc.tile_pool(name="wcol", bufs=2) as wc_p, \
         tc.tile_pool(name="wv", bufs=1) as wv_p:

        xsT = [xsT_p.tile([128, L], F32R, tag="xsT", name=f"xsT{i}")
               for i in range(8)]
        vaug = [vaug_p.tile([128, H * (HD + 1)], F32R, tag="vaug",
                            name=f"vaug{i}") for i in range(8)]

        # ---- phase 0: load xs, build xsT ---------------------------------
        with tc.tile_pool(name="xs_nat", bufs=8) as xsn_p, \
             tc.tile_pool(name="tp_ps", bufs=4, space="PSUM") as tp_ps:
            xs_nat = []
            for r in range(8):
                t = xsn_p.tile([128, D], F32R, tag="xsn", name=f"xsn{r}")
                nc.sync.dma_start(out=t[:],
                                  in_=xs[r * 128:(r + 1) * 128, :].bitcast(F32R))
                xs_nat.append(t)
            for g in range(2):        # g first: start after half the DMA
                for c in range(8):
                    pt = tp_ps.tile([128, 512], F32R, tag="tp", name="tp")
                    for k in range(4):
                        r = g * 4 + k
                        nc.tensor.transpose(
                            pt[:, k * 128:(k + 1) * 128],
                            xs_nat[r][:, c * 128:(c + 1) * 128],
                            idr[:],
                        )
                    nc.vector.tensor_copy(
                        out=xsT[c][:, g * 512:(g + 1) * 512], in_=pt[:])

        for l in range(8):
            dst = vaug[l][:].rearrange("p (h e) -> p h e", e=HD + 1)
            nc.vector.tensor_copy(out=dst[:, :, HD:HD + 1],
                                  in_=ones16[:].unsqueeze(2))

        # ---------- emission helpers --------------------------------------
        def emit_qk_tile(m, dest):
            """qkT row-tile m (dims m*128..) -> dest tile [128, L]."""
            wcol = wc_p.tile([128, 8, 128], F32R, tag="w", name="wcol")
            nc.sync.dma_start(
                out=wcol[:],
                in_=wqkv[:, m * 128:(m + 1) * 128]
                .rearrange("(r p) m -> p r m", p=128).bitcast(F32R),
            )
            units = []
            for half in range(2):
                def unit(half=half, wcol=wcol):
                    ps = proj_ps.tile([128, 512], F32, tag="proj", name="ps")
                    for r in range(8):
                        nc.tensor.matmul(
                            ps[:], wcol[:, r, :],
                            xsT[r][:, half * 512:(half + 1) * 512],
                            start=(r == 0), stop=(r == 7),
                        )
                    nc.vector.tensor_scalar_add(
                        out=dest[:, half * 512:(half + 1) * 512],
                        in0=ps[:], scalar1=bqk[:, m:m + 1])
                units.append(unit)
            return units

        def emit_k_tile(m, dest0, dest1):
            """k row-tile m -> two zero-padded per-head tiles.

            dest0 keeps rows 0:64 (head 2p) and zeroes rows 64:128;
            dest1 keeps rows 64:128 (head 2p+1) and zeroes rows 0:64, so
            score matmuls can contract K=128 from base partition 0.
            """
            wcol = wc_p.tile([128, 8, 128], F32R, tag="w", name="wcol")
            nc.sync.dma_start(
                out=wcol[:],
                in_=wqkv[:, m * 128:(m + 1) * 128]
                .rearrange("(r p) m -> p r m", p=128).bitcast(F32R),
            )
            units = []
            for half in range(2):
                def unit(half=half, wcol=wcol):
                    ps = proj_ps.tile([128, 512], F32, tag="proj", name="ps")
                    for r in range(8):
                        nc.tensor.matmul(
                            ps[:], wcol[:, r, :],
                            xsT[r][:, half * 512:(half + 1) * 512],
                            start=(r == 0), stop=(r == 7),
                        )
                    for dest, mc in ((dest0, 0), (dest1, 1)):
                        nc.vector.tensor_scalar(
                            out=dest[:, half * 512:(half + 1) * 512],
                            in0=ps[:], scalar1=bqk[:, m:m + 1],
                            scalar2=pmask[:, mc:mc + 1],
                            op0=mybir.AluOpType.add,
                            op1=mybir.AluOpType.mult)
                units.append(unit)
            return units

        def emit_v_half(q):
            """v half q (heads 8q..8q+7) into vaug tiles; one unit per l."""
            wv = wv_p.tile([128, 8, 512], F32R, tag="wv", name="wv")
            nc.sync.dma_start(
                out=wv[:],
                in_=wqkv[:, NQK + q * 512:NQK + (q + 1) * 512]
                .rearrange("(r p) n -> p r n", p=128).bitcast(F32R),
            )
            units = []
            for l in range(8):
                def unit(l=l, wv=wv):
                    ps = proj_ps.tile([128, 512], F32, tag="proj", name="vps")
                    for r in range(8):
                        nc.tensor.matmul(
                            ps[:], xsT[r][:, l * 128:(l + 1) * 128],
                            wv[:, r, :],
                            start=(r == 0), stop=(r == 7),
                        )
                    dst = vaug[l][:].rearrange("p (h e) -> p h e", e=HD + 1)
                    nc.vector.tensor_tensor(
                        out=dst[:, q * 8:(q + 1) * 8, 0:HD],
                        in0=ps[:].rearrange("p (h e) -> p h e", e=HD),
                        in1=bv_bc[:].rearrange("p (h e) -> p h e", e=HD)[
                            :, q * 8:(q + 1) * 8, :],
                        op=mybir.AluOpType.add,
                    )
                units.append(unit)
            return units

        # ---- q/k projection for pair 0, then v quarter 0 -----------------
        qk_tiles = {}
        fillers_iters = {}
        qk_tiles[0] = (qkT_p.tile([128, L], F32R, tag="qkT", name="qt0"),
                       qkT_p.tile([128, L], F32R, tag="qkT", name="kt0a"),
                       qkT_p.tile([128, L], F32R, tag="qkT", name="kt0b"))
        for u in emit_qk_tile(0, qk_tiles[0][0]):
            u()
        for u in emit_k_tile(8, qk_tiles[0][1], qk_tiles[0][2]):
            u()
        for u in emit_v_half(0):
            u()

        # ---- attention heads with interleaved fillers --------------------
        with tc.tile_pool(name="expT", bufs=4) as exp_p, \
             tc.tile_pool(name="craw", bufs=2) as craw_p, \
             tc.tile_pool(name="srow", bufs=1) as srow_p, \
             tc.tile_pool(name="rbc", bufs=2) as rbc_p, \
             tc.tile_pool(name="s_ps", bufs=2, space="PSUM") as s_ps, \
             tc.tile_pool(name="c_ps", bufs=1, space="PSUM") as c_ps:

            for h in range(H):
                p = h // 2
                po = (h % 2) * HD
                qt = qk_tiles[p][0]
                ktp = qk_tiles[p][1 + (h % 2)]

                # fillers: emitted once per pair, consumed across two heads
                if h % 2 == 0:
                    fillers = []
                    if p < 1:
                        fillers += emit_v_half(p + 1)
                    if p < 7:
                        nxt = (qkT_p.tile([128, L], F32R, tag="qkT",
                                          name=f"qt{p+1}"),
                               qkT_p.tile([128, L], F32R, tag="qkT",
                                          name=f"kt{p+1}a"),
                               qkT_p.tile([128, L], F32R, tag="qkT",
                                          name=f"kt{p+1}b"))
                        qk_tiles[p + 1] = nxt
                        fillers += emit_qk_tile(p + 1, nxt[0])
                        fillers += emit_k_tile(9 + p, nxt[1], nxt[2])
                    fillers_iters[p] = iter(fillers)
                fillers_iter = fillers_iters[p]

                cps = c_ps.tile([128, L], F32, tag="cps", name="cps")
                for c in range(8):
                    sps = s_ps.tile([128, L], F32, tag="sps", name="sps")
                    for half in range(2):
                        nc.tensor.matmul(
                            sps[:, half * 512:(half + 1) * 512],
                            ktp[:, c * 128:(c + 1) * 128],
                            qt[:, half * 512:(half + 1) * 512],
                            start=True, stop=True,
                        )
                    et = exp_p.tile([128, L], F32R, tag="expT", name="et")
                    nc.scalar.activation(out=et[:], in_=sps[:], func=AF.Exp,
                                         scale=SCALE)
                    u = next(fillers_iter, None)
                    if u is not None:
                        u()
                    for half in range(2):
                        nc.tensor.matmul(
                            cps[0:HD + 1, half * 512:(half + 1) * 512],
                            vaug[c][:, h * (HD + 1):(h + 1) * (HD + 1)],
                            et[:, half * 512:(half + 1) * 512],
                            start=(c == 0), stop=(c == 7),
                        )
                # free the ctx psum bank right away; normalize asynchronously
                craw = craw_p.tile([HD + 1, L], F32, tag="craw", name="craw")
                nc.vector.tensor_copy(out=craw[:], in_=cps[0:HD + 1, :])
                rec = srow_p.tile([1, L], F32, tag="srow", name="rec")
                nc.vector.reciprocal(out=rec[:], in_=craw[HD:HD + 1, :])
                rbc = rbc_p.tile([HD, L], F32, tag="rbc", name="rbc")
                nc.gpsimd.partition_broadcast(rbc[:], rec[:])
                nc.vector.tensor_mul(
                    ctxT[h // 2][po:po + HD, :], craw[0:HD, :], rbc[:])
                if h % 2 == 1:
                    for u in fillers_iter:   # drain leftovers
                        u()
                    del qk_tiles[p]
                    del fillers_iters[p]

    # ---- phase 3: out = ctxT.T-contract @ Wout + bout --------------------
    with tc.tile_pool(name="wout", bufs=8) as wo_p, \
         tc.tile_pool(name="o_sb", bufs=4) as o_sb:
        wo = []
        for r in range(8):
            t = wo_p.tile([128, D], F32R, tag="wo", name=f"wo{r}")
            nc.sync.dma_start(
                out=t[:], in_=wout[r * 128:(r + 1) * 128, :].bitcast(F32R))
            wo.append(t)
        for l in range(8):
            for half in range(2):
                ps = proj_ps.tile([128, 512], F32, tag="proj", name="ops")
                for r in range(8):
                    nc.tensor.matmul(
                        ps[:], ctxT[r][:, l * 128:(l + 1) * 128],
                        wo[r][:, half * 512:(half + 1) * 512],
                        start=(r == 0), stop=(r == 7),
                    )
                osb = o_sb.tile([128, 512], F32, tag="osb", name="osb")
                nc.vector.tensor_tensor(
                    out=osb[:], in0=ps[:],
                    in1=bout_bc[:, half * 512:(half + 1) * 512],
                    op=mybir.AluOpType.add)
                nc.sync.dma_start(
                    out=out[l * 128:(l + 1) * 128,
                            half * 512:(half + 1) * 512],
                    in_=osb[:],
                )


def get_nc():
    if "nc" not in _CACHE:
        _CACHE["nc"] = _build()
    return _CACHE["nc"]


def make_in_maps(x, Wqkv, bqkv, Wout, bout):
    """Shard: core i -> (batch i//N_SEG, segment i%N_SEG), dilated tokens."""
    x = np.asarray(x, dtype=np.float32)
    Wqkv = np.ascontiguousarray(np.asarray(Wqkv, dtype=np.float32))
    bqkv = np.ascontiguousarray(np.asarray(bqkv, dtype=np.float32))
    Wout = np.ascontiguousarray(np.asarray(Wout, dtype=np.float32))
    bout = np.ascontiguousarray(np.asarray(bout, dtype=np.float32))
    in_maps = []
    for i in range(N_CORES):
        b, seg = divmod(i, N_SEG)
        xs = np.ascontiguousarray(
            x[b, seg * SEGMENT:(seg + 1) * SEGMENT:DILATION, :])
        in_maps.append({"xs": xs, "Wqkv": Wqkv, "bqkv": bqkv,
                        "Wout": Wout, "bout": bout})
    return in_maps


def unshard(results):
    out = np.empty((B, N_SEG * L, D), dtype=np.float32)
    for i in range(N_CORES):
        b, seg = divmod(i, N_SEG)
        out[b, seg * L:(seg + 1) * L, :] = results[i]["out"]
    return out


def kernel(x, Wqkv, bqkv, Wout, bout):
    nc = get_nc()
    in_maps = make_in_maps(x, Wqkv, bqkv, Wout, bout)
    res = bass_utils.run_bass_kernel_spmd(nc, in_maps,
                                          core_ids=list(range(N_CORES)))
    return unshard(res.results)



# revision 5
# speedup vs baseline: 1.0045x; 1.0045x over previous
"""Dilated-attention (segmented FlashMHA) for Trainium2, 8-core data parallel.

Problem (hardcoded): x [2, 8192, 1024], SEGMENT=2048, DILATION=2, 16 heads.
Each (batch, segment) pair is an independent attention problem over the
L = 1024 dilated tokens; there are exactly B * n_seg = 2 * 4 = 8 of them,
one per NeuronCore.  Weights are replicated.

v2 (bf16): tolerance is 2e-2 and an all-bf16 matmul pipeline measures
~6e-3 max-rel error in numpy, so every matmul runs bf16 (fp32 PSUM
accumulate).  The host pre-casts weights/activations to bf16 and
pre-transposes xs, which removes the on-device transpose phase entirely
and halves HBM traffic:

  host:  xsT [D, L] bf16, wqk [2048, 1024] bf16 (per-m row-tiles of
         Wqkv[:, :2048], contraction-major), wv [256, 4096] bf16,
         wout [1024, 1024] bf16, biases fp32.

Per-core kernel (all matmul inputs bf16):
  qkT  = Wqkv[:, :2048].T @ xsT  (+bias)   q/k kept transposed [dim, token]
  v    = xsT.T-contract @ Wv    (+bias)    natural [token, dim], stored
                                           head-blocked with a ones column
                                           appended per head (v_aug)
  per head:  sT = k.q (transposed scores), eT = exp(sT/8) via ACT (bf16),
             ctxT_aug = sum_ck v_aug.T-contract @ eT  ([65, lq]; row 64 is
             the softmax denominator via the ones column),
             ctxT = ctxT_aug[0:64] * recip(denom) (DVE + gpsimd bcast)
  out  = ctxT.T-contract @ Wout + bout     natural layout, fp32 out

kT is stored one head per zero-padded 128-row tile (dead rows zeroed by a
masked tensor_scalar eviction) so score matmuls contract K=128 from base
partition 0 (partial-K measured ~2.2x slower on HW).  All weights are
DMA'd up-front (everything fits SBUF in bf16); the per-head attention
loop interleaves projection units as PE fillers; v-half-0 units are
deferred into head 0 so attention starts after only 4 projection units.
"""

from contextlib import ExitStack

import numpy as np
import ml_dtypes

from concourse import bacc, bass_utils, mybir, tile
from concourse._compat import with_exitstack

F32 = mybir.dt.float32
BF16 = mybir.dt.bfloat16
AF = mybir.ActivationFunctionType
NPBF16 = ml_dtypes.bfloat16

B = 2
S = 8192
D = 1024
SEGMENT = 2048
DILATION = 2
N_SEG = S // SEGMENT          # 4
L = SEGMENT // DILATION       # 1024 tokens per (b, seg)
H = 16
HD = 64
NQK = 2048
SCALE = 0.125                 # 1 / sqrt(HD)
N_CORES = 8

_CACHE = {}


def _build(n_cores=N_CORES, loop_n=1):
    nc = bacc.Bacc("TRN2", debug=False, num_devices=n_cores)

    xsT_d = nc.dram_tensor("xsT", (D, L), BF16, kind="ExternalInput")
    wqk_d = nc.dram_tensor("wqk", (NQK, D), BF16, kind="ExternalInput")
    wv_d = nc.dram_tensor("wv", (2 * 128, 8 * 512), BF16, kind="ExternalInput")
    wout_d = nc.dram_tensor("wout", (D, D), BF16, kind="ExternalInput")
    bqkv_d = nc.dram_tensor("bqkv", (3 * D,), F32, kind="ExternalInput")
    bout_d = nc.dram_tensor("bout", (D,), F32, kind="ExternalInput")
    out_d = nc.dram_tensor("out", (L, D), F32, kind="ExternalOutput")

    with tile.TileContext(nc) as tc:
        if loop_n > 1:
            with tc.For_i(0, loop_n, 1):
                _emit(tc, out_d.ap(), xsT_d.ap(), wqk_d.ap(), wv_d.ap(),
                      wout_d.ap(), bqkv_d.ap(), bout_d.ap())
        else:
            _emit(tc, out_d.ap(), xsT_d.ap(), wqk_d.ap(), wv_d.ap(),
                  wout_d.ap(), bqkv_d.ap(), bout_d.ap())
    nc.compile()
    return nc


@with_exitstack
def _emit(ctx: ExitStack, tc, out, xsT_dram, wqk_dram, wv_dram, wout_dram,
          bqkv, bout):
    nc = tc.nc

    const_p = ctx.enter_context(tc.tile_pool(name="const", bufs=1))
    ctxT_p = ctx.enter_context(tc.tile_pool(name="ctxT", bufs=8))
    # PSUM pool shared by all projection matmuls (qk / v / out): 2 banks
    proj_ps = ctx.enter_context(
        tc.tile_pool(name="proj_ps", bufs=2, space="PSUM"))

    # f32 constants
    bqk = const_p.tile([128, 16], F32)
    nc.sync.dma_start(out=bqk[:], in_=bqkv[0:NQK].rearrange("(c p) -> p c", p=128))
    # bias tiles broadcast across partitions (0-stride DRAM read); scalar
    # queue keeps them off the critical gpsimd weight queue
    bv_bc = const_p.tile([128, D], F32)
    nc.scalar.dma_start(out=bv_bc[:],
                        in_=bqkv[NQK:3 * D].partition_broadcast(128))
    bout_bc = const_p.tile([128, D], F32)
    nc.scalar.dma_start(out=bout_bc[:], in_=bout.partition_broadcast(128))
    ones16 = const_p.tile([128, 16], F32)
    nc.vector.memset(ones16[:], 1.0)
    # partition masks: col 0 selects rows 0:64, col 1 selects rows 64:128
    pmask = const_p.tile([128, 2], F32)
    nc.vector.memset(pmask[0:HD, 0:1], 1.0)
    nc.vector.memset(pmask[HD:128, 0:1], 0.0)
    nc.vector.memset(pmask[0:HD, 1:2], 0.0)
    nc.vector.memset(pmask[HD:128, 1:2], 1.0)

    ctxT = [ctxT_p.tile([128, L], BF16, tag="ctxT", name=f"ctxT{i}")
            for i in range(8)]

    with tc.tile_pool(name="xsT", bufs=8) as xsT_p, \
         tc.tile_pool(name="vaug", bufs=8) as vaug_p, \
         tc.tile_pool(name="qkT", bufs=6) as qkT_p, \
         tc.tile_pool(name="wqk", bufs=16) as wqk_p, \
         tc.tile_pool(name="wv", bufs=2) as wv_p, \
         tc.tile_pool(name="wout", bufs=8) as wo_p:

        # ---- up-front DMA of all inputs (weights fully resident) --------
        xsT = []
        for r in range(8):
            t = xsT_p.tile([128, L], BF16, tag="xsT", name=f"xsT{r}")
            nc.sync.dma_start(out=t[:], in_=xsT_dram[r * 128:(r + 1) * 128, :])
            xsT.append(t)
        wqk = [wqk_p.tile([128, D], BF16, tag="w", name=f"wqk{m}")
               for m in range(16)]
        wv = [wv_p.tile([128, 8 * 512], BF16, tag="wv", name=f"wv{q}")
              for q in range(2)]
        wo = []

        def dma_wqk(m):
            nc.gpsimd.dma_start(out=wqk[m][:],
                                in_=wqk_dram[m * 128:(m + 1) * 128, :])

        def dma_wv(q):
            nc.gpsimd.dma_start(out=wv[q][:],
                                in_=wv_dram[q * 128:(q + 1) * 128, :])

        # order: pair-0 weights first, then v halves, then the rest
        dma_wqk(0)
        dma_wqk(8)
        dma_wv(0)
        dma_wqk(1)
        dma_wqk(9)
        dma_wv(1)
        for p in range(2, 8):
            dma_wqk(p)
            dma_wqk(8 + p)
        for r in range(8):
            t = wo_p.tile([128, D], BF16, tag="wo", name=f"wo{r}")
            nc.scalar.dma_start(
                out=t[:], in_=wout_dram[r * 128:(r + 1) * 128, :])
            wo.append(t)

        vaug = [vaug_p.tile([128, H * (HD + 1)], BF16, tag="vaug",
                            name=f"vaug{i}") for i in range(8)]
        for l in range(8):
            dst = vaug[l][:].rearrange("p (h e) -> p h e", e=HD + 1)
            nc.vector.tensor_copy(out=dst[:, :, HD:HD + 1],
                                  in_=ones16[:].unsqueeze(2))

        # ---------- emission helpers --------------------------------------
        def emit_qk_half(m, dest, half):
            """qkT row-tile m (dims m*128..), token half -> dest[:, half]."""
            def unit():
                ps = proj_ps.tile([128, 512], F32, tag="proj", name="ps")
                for r in range(8):
                    nc.tensor.matmul(
                        ps[:], wqk[m][:, r * 128:(r + 1) * 128],
                        xsT[r][:, half * 512:(half + 1) * 512],
                        start=(r == 0), stop=(r == 7),
                    )
                nc.vector.tensor_scalar_add(
                    out=dest[:, half * 512:(half + 1) * 512],
                    in0=ps[:], scalar1=bqk[:, m:m + 1])
            return unit

        def emit_k_half(m, dest0, dest1, half):
            """k row-tile m, token half -> two zero-padded per-head tiles.

            dest0 keeps rows 0:64 (head 2p) and zeroes rows 64:128;
            dest1 keeps rows 64:128 (head 2p+1) and zeroes rows 0:64, so
            score matmuls contract K=128 from base partition 0.
            """
            def unit():
                ps = proj_ps.tile([128, 512], F32, tag="proj", name="ps")
                for r in range(8):
                    nc.tensor.matmul(
                        ps[:], wqk[m][:, r * 128:(r + 1) * 128],
                        xsT[r][:, half * 512:(half + 1) * 512],
                        start=(r == 0), stop=(r == 7),
                    )
                for dest, mc in ((dest0, 0), (dest1, 1)):
                    nc.vector.tensor_scalar(
                        out=dest[:, half * 512:(half + 1) * 512],
                        in0=ps[:], scalar1=bqk[:, m:m + 1],
                        scalar2=pmask[:, mc:mc + 1],
                        op0=mybir.AluOpType.add,
                        op1=mybir.AluOpType.mult)
            return unit

        def emit_v_unit(q, l):
            """v half q (heads 8q..8q+7), token chunk l -> vaug[l]."""
            def unit():
                ps = proj_ps.tile([128, 512], F32, tag="proj", name="vps")
                for r in range(8):
                    nc.tensor.matmul(
                        ps[:], xsT[r][:, l * 128:(l + 1) * 128],
                        wv[q][:, r * 512:(r + 1) * 512],
                        start=(r == 0), stop=(r == 7),
                    )
                dst = vaug[l][:].rearrange("p (h e) -> p h e", e=HD + 1)
                nc.vector.tensor_tensor(
                    out=dst[:, q * 8:(q + 1) * 8, 0:HD],
                    in0=ps[:].rearrange("p (h e) -> p h e", e=HD),
                    in1=bv_bc[:].rearrange("p (h e) -> p h e", e=HD)[
                        :, q * 8:(q + 1) * 8, :],
                    op=mybir.AluOpType.add,
                )
            return unit

        # ---- prelude: q pair0 (both halves), k pair0 half 0, vaug[0] ----
        qk_tiles = {}
        fillers_iters = {}
        qk_tiles[0] = (qkT_p.tile([128, L], BF16, tag="qkT", name="qt0"),
                       qkT_p.tile([128, L], BF16, tag="qkT", name="kt0a"),
                       qkT_p.tile([128, L], BF16, tag="qkT", name="kt0b"))
        emit_qk_half(0, qk_tiles[0][0], 0)()
        emit_qk_half(0, qk_tiles[0][0], 1)()
        emit_k_half(8, qk_tiles[0][1], qk_tiles[0][2], 0)()
        emit_v_unit(0, 0)()

        # ---- attention heads with interleaved fillers --------------------
        with tc.tile_pool(name="expT", bufs=4) as exp_p, \
             tc.tile_pool(name="craw", bufs=2) as craw_p, \
             tc.tile_pool(name="srow", bufs=2) as srow_p, \
             tc.tile_pool(name="rbc", bufs=2) as rbc_p, \
             tc.tile_pool(name="s_ps", bufs=2, space="PSUM") as s_ps, \
             tc.tile_pool(name="c_ps", bufs=1, space="PSUM") as c_ps:

            for h in range(H):
                p = h // 2
                po = (h % 2) * HD
                qt = qk_tiles[p][0]
                ktp = qk_tiles[p][1 + (h % 2)]

                # fillers: emitted per head-pair, consumed across two heads
                if h == 0:
                    # deferred k half 1 (needed by sps c=4) and the rest of
                    # v half 0 (vaug[c] needed by ctx chunk c)
                    fillers = [emit_v_unit(0, 1), emit_v_unit(0, 2),
                               emit_v_unit(0, 3),
                               emit_k_half(8, qk_tiles[0][1], qk_tiles[0][2], 1),
                               emit_v_unit(0, 4), emit_v_unit(0, 5),
                               emit_v_unit(0, 6), emit_v_unit(0, 7)]
                    fillers_iters[0] = iter(fillers)
                elif h == 1:
                    fillers = [emit_v_unit(1, l) for l in range(8)]
                    nxt = (qkT_p.tile([128, L], BF16, tag="qkT", name="qt1"),
                           qkT_p.tile([128, L], BF16, tag="qkT", name="kt1a"),
                           qkT_p.tile([128, L], BF16, tag="qkT", name="kt1b"))
                    qk_tiles[1] = nxt
                    fillers += [emit_qk_half(1, nxt[0], 0),
                                emit_qk_half(1, nxt[0], 1),
                                emit_k_half(9, nxt[1], nxt[2], 0),
                                emit_k_half(9, nxt[1], nxt[2], 1)]
                    fillers_iters[0] = iter(fillers)
                elif h % 2 == 0 and p < 7:
                    nxt = (qkT_p.tile([128, L], BF16, tag="qkT",
                                      name=f"qt{p+1}"),
                           qkT_p.tile([128, L], BF16, tag="qkT",
                                      name=f"kt{p+1}a"),
                           qkT_p.tile([128, L], BF16, tag="qkT",
                                      name=f"kt{p+1}b"))
                    qk_tiles[p + 1] = nxt
                    fillers = [emit_qk_half(p + 1, nxt[0], 0),
                               emit_qk_half(p + 1, nxt[0], 1),
                               emit_k_half(9 + p, nxt[1], nxt[2], 0),
                               emit_k_half(9 + p, nxt[1], nxt[2], 1)]
                    fillers_iters[p] = iter(fillers)
                elif h % 2 == 0:
                    fillers_iters[p] = iter([])
                fillers_iter = fillers_iters[p]

                cps = c_ps.tile([128, L], F32, tag="cps", name="cps")
                for c in range(8):
                    sps = s_ps.tile([128, L], F32, tag="sps", name="sps")
                    for half in range(2):
                        nc.tensor.matmul(
                            sps[:, half * 512:(half + 1) * 512],
                            ktp[:, c * 128:(c + 1) * 128],
                            qt[:, half * 512:(half + 1) * 512],
                            start=True, stop=True,
                        )
                    et = exp_p.tile([128, L], BF16, tag="expT", name="et")
                    nc.scalar.activation(out=et[:], in_=sps[:], func=AF.Exp,
                                         scale=SCALE)
                    u = next(fillers_iter, None)
                    if u is not None:
                        u()
                    for half in range(2):
                        nc.tensor.matmul(
                            cps[0:HD + 1, half * 512:(half + 1) * 512],
                            vaug[c][:, h * (HD + 1):(h + 1) * (HD + 1)],
                            et[:, half * 512:(half + 1) * 512],
                            start=(c == 0), stop=(c == 7),
                        )
                # free the ctx psum bank right away; normalize asynchronously
                craw = craw_p.tile([HD + 1, L], F32, tag="craw", name="craw")
                nc.vector.tensor_copy(out=craw[:], in_=cps[0:HD + 1, :])
                rec = srow_p.tile([1, L], F32, tag="srow", name="rec")
                nc.vector.reciprocal(out=rec[:], in_=craw[HD:HD + 1, :])
                rbc = rbc_p.tile([HD, L], F32, tag="rbc", name="rbc")
                nc.gpsimd.partition_broadcast(rbc[:], rec[:])
                nc.vector.tensor_mul(
                    ctxT[h // 2][po:po + HD, :], craw[0:HD, :], rbc[:])
                if h % 2 == 1:
                    for u in fillers_iter:   # drain leftovers
                        u()
                    del qk_tiles[p]
                    del fillers_iters[p]

        # ---- phase 3: out = ctxT.T-contract @ Wout + bout ----------------
        with tc.tile_pool(name="o_sb", bufs=4) as o_sb:
            for l in range(8):
                for half in range(2):
                    ps = proj_ps.tile([128, 512], F32, tag="proj", name="ops")
                    for r in range(8):
                        nc.tensor.matmul(
                            ps[:], ctxT[r][:, l * 128:(l + 1) * 128],
                            wo[r][:, half * 512:(half + 1) * 512],
                            start=(r == 0), stop=(r == 7),
                        )
                    osb = o_sb.tile([128, 512], F32, tag="osb", name="osb")
                    nc.vector.tensor_tensor(
                        out=osb[:], in0=ps[:],
                        in1=bout_bc[:, half * 512:(half + 1) * 512],
                        op=mybir.AluOpType.add)
                    nc.sync.dma_start(
                        out=out[l * 128:(l + 1) * 128,
                                half * 512:(half + 1) * 512],
                        in_=osb[:],
                    )


def get_nc():
    if "nc" not in _CACHE:
        _CACHE["nc"] = _build()
    return _CACHE["nc"]


def make_in_maps(x, Wqkv, bqkv, Wout, bout):
    """Shard: core i -> (batch i//N_SEG, segment i%N_SEG), dilated tokens.

    Host-side prep: cast to bf16, pre-transpose xs, and lay weights out
    contraction-major so every DMA is a contiguous [128, N] row-tile.
    """
    x = np.asarray(x, dtype=np.float32)
    Wqkv = np.asarray(Wqkv, dtype=np.float32)
    bqkv = np.ascontiguousarray(np.asarray(bqkv, dtype=np.float32))
    Wout = np.asarray(Wout, dtype=np.float32)
    bout = np.ascontiguousarray(np.asarray(bout, dtype=np.float32))

    wqkv_bf = Wqkv.astype(NPBF16)
    # [16 m, 128 p, 8 r, 128 c] -> [2048, 1024]: row-tile m is contiguous
    wqk = np.ascontiguousarray(
        wqkv_bf[:, :NQK].reshape(8, 128, 16, 128).transpose(2, 1, 0, 3)
    ).reshape(NQK, D)
    # [2 q, 128 p, 8 r, 512 c] -> [256, 4096]
    wv = np.ascontiguousarray(
        wqkv_bf[:, NQK:].reshape(8, 128, 2, 512).transpose(2, 1, 0, 3)
    ).reshape(256, 4096)
    wout = np.ascontiguousarray(Wout.astype(NPBF16))

    in_maps = []
    for i in range(N_CORES):
        b, seg = divmod(i, N_SEG)
        xs = x[b, seg * SEGMENT:(seg + 1) * SEGMENT:DILATION, :]
        xsT = np.ascontiguousarray(xs.T.astype(NPBF16))
        in_maps.append({"xsT": xsT, "wqk": wqk, "wv": wv, "wout": wout,
                        "bqkv": bqkv, "bout": bout})
    return in_maps


def unshard(results):
    out = np.empty((B, N_SEG * L, D), dtype=np.float32)
    for i in range(N_CORES):
        b, seg = divmod(i, N_SEG)
        out[b, seg * L:(seg + 1) * L, :] = results[i]["out"]
    return out


def kernel(x, Wqkv, bqkv, Wout, bout):
    nc = get_nc()
    in_maps = make_in_maps(x, Wqkv, bqkv, Wout, bout)
    res = bass_utils.run_bass_kernel_spmd(nc, in_maps,
                                          core_ids=list(range(N_CORES)))
    return unshard(res.results)
